# revision 20
# baseline (speedup 1.0000x reference)
"""Trainium2 Bass kernel for nn_DifferentiableADF (angular distribution function).

Computes: for M=500k angle triplets over xyz[8,512,3], the Gaussian-smeared
180-bin histogram of bond angles, normalized to sum 1.

Strategy (8 cores, data-parallel over angles):
  - angle_list sharded M/8 per core, each angle packed host-side into ONE
    int32 word (f | i<<3 | c<<12 | j<<21); unpacked on the DVE into three
    int16 flat-table indices t = f*512 + atom. This is the only large
    per-call transfer (256 KiB/core) -- the axon tunnel is the bottleneck
    (~90 ms fixed RPC + ~87 MB/s), so input bytes are minimized.
  - xyz ships sharded: core r carries only frame r's atoms as [3, 512]
    (6 KiB); an on-device AllGather assembles the full flat table, then
    0-stride DMAs broadcast it to the [128, 4096] gather table (row p =
    coord p%3).
  - mask / smearing matrix / coefficients are inline_tensor Consts baked
    into the NEFF (loaded to HBM once at model load, zero per-call cost).
  - per chunk: GPSIMD ap_gather fetches coords; a contiguous-block DMA
    repack aligns the stream to compute partitions. Bond vectors + dots on
    DVE, arccos via A&S 4.4.46 polynomial, fast-Gauss-transform moment
    accumulation: theta -> nearest fine bin q, moments (1, eps, eps^2,
    eps^3) scattered into bins via a digit-split one-hot matmul on the PE
    (PSUM accumulates across all chunks).
  - AllReduce of the [64,12] moment block, then a tiny matmul against a
    precomputed Hermite-derivative matrix reconstructs the exact smeared
    histogram; normalized on device. All cores produce identical output.
  - Execution: a cached jax.jit(shard_map) over the bass_exec custom call
    (built once per process); per-call cost is one execute RPC with the
    packed inputs pipelined in.
"""

import math
import os
import sys
from contextlib import ExitStack

import numpy as np

sys.path.insert(0, "/opt/trn_rl_repo")

import concourse.bass as bass  # noqa: E402
import concourse.tile as tile  # noqa: E402
from concourse.tile import add_dep_helper  # noqa: E402
from concourse import bacc, mybir  # noqa: E402
from concourse._compat import with_exitstack  # noqa: E402

F32 = mybir.dt.float32
I32 = mybir.dt.int32
I16 = mybir.dt.int16
AF = mybir.ActivationFunctionType
OP = mybir.AluOpType

# ---------------- problem constants ----------------
N_FRAMES = 8
N_ATOMS = 512
N_ANGLES = 500_000
NBINS = 180
H = 180.0 / 179.0  # bin spacing == fine-grid spacing
N_CORES = 8
PER_CORE = N_ANGLES // N_CORES  # 62500

QL = 8   # low digit of fine-bin index
QH = 24  # high digit (8*24 = 192 >= 180 bins; q in [0,191] all valid rows)
PMOM = 4  # moments kept: eps^0..eps^3
KFLAT = QL * PMOM * QH  # 768 = 6*128
DEG = 180.0 / math.pi

# layout: angle slot s = ((p*CHUNKS + k)*C + j)  p: partition, k: chunk, j: col
CHUNKS = 8
C = 64  # must be multiple of 16 (contiguous-block repack needs 3C % 48 == 0)
SLOTS = 128 * CHUNKS * C  # 65536 >= 62500

# Abramowitz & Stegun 4.4.46: arccos(x) = sqrt(1-x) * sum a_k x^k, x in [0,1]
ACOS_COEF = [
    1.5707963050, -0.2145988016, 0.0889789874, -0.0501743046,
    0.0308918810, -0.0170881256, 0.0066700901, -0.0012624911,
]


def build_amat() -> np.ndarray:
    """A[(ql*PMOM+pm)*QH+qh, b] = g^(pm)(c_q - o_b)/pm!  with g = exp(-x^2/2)."""
    q = np.arange(QL * QH, dtype=np.float64)
    b = np.arange(NBINS, dtype=np.float64)
    d = q[:, None] * H - b[None, :] * H  # [192, 180]
    g0 = np.exp(-0.5 * d * d)
    derivs = [g0, -d * g0, (d * d - 1.0) / 2.0 * g0, (3.0 * d - d**3) / 6.0 * g0]
    a = np.zeros((KFLAT, NBINS), dtype=np.float64)
    for qi in range(QL * QH):
        ql, qh = qi % QL, qi // QL
        for pm in range(PMOM):
            a[(ql * PMOM + pm) * QH + qh, :] = derivs[pm][qi, :]
    return a.astype(np.float32)


def build_mask_math(per: int, chunks: int, cols: int) -> np.ndarray:
    """Validity mask in the post-repack math layout (p-minor slot order).

    original slot s' = (k*cols + j)*128 + p is valid iff s' < per; math slot
    (p'=16g+w', k, 16*jj + w0) maps to (p=16g+w0, k, j=(cols//16)*w' + jj)."""
    jj = cols // 16
    k_, j_, p_ = np.meshgrid(
        np.arange(chunks), np.arange(cols), np.arange(128), indexing="ij"
    )
    valid = ((k_ * cols + j_) * 128 + p_ < per).astype(np.float32)  # [k, j, p]
    v = valid.transpose(2, 0, 1)  # [p, k, j]
    m = v.reshape(8, 16, chunks, 16, jj)  # [g, w0, k, w', jj]
    mm = np.transpose(m, (0, 3, 2, 4, 1))  # [g, w', k, jj, w0]
    return np.ascontiguousarray(mm.reshape(128, chunks, cols))


def coefs_tile() -> np.ndarray:
    row = np.zeros(12, dtype=np.float32)
    row[:8] = np.array([c * DEG for c in ACOS_COEF], dtype=np.float32)
    row[8] = 1e-30
    return np.broadcast_to(row, (128, 12)).copy()


def amat_inline() -> np.ndarray:
    """amat pre-rearranged to the SBUF layout [128, 6, NBINS]:
    element (p, cq, b) = amat[cq*128 + p, b]."""
    a = build_amat()  # [768, 180]
    return np.ascontiguousarray(a.reshape(6, 128, NBINS).transpose(1, 0, 2))


@with_exitstack
def adf_kernel(ctx: ExitStack, tc: tile.TileContext, outs, ins, raw, consts,
               per=None):
    nc = tc.nc
    xyzt_sb, idxs16_raw, gath_raw = raw
    blob = ins["blob"]      # [128, CHUNKS*C+12] int32: packed angles + xyz bits
    nbl = blob.shape[1]
    al = blob[:, 0 : nbl - 12].rearrange(
        "p (k c) -> p k c", k=CHUNKS
    )                       # [128, CHUNKS, C] int32 packed (f|i<<3|c<<12|j<<21)
    # xyz slice (f32 bits): word (c3, a=hi*128+lo) is at blob[lo, nbl-12+c3*4+hi]
    xyz_words = blob[:, nbl - 12 : nbl].bitcast(F32)
    mask_in, amat_in, coefs_in = consts  # inline Const DRAM handles
    out = outs["out"]       # [180] f32

    chunks, cc = al.shape[1], al.shape[2]

    const_pool = ctx.enter_context(tc.tile_pool(name="const", bufs=1))
    pool = ctx.enter_context(tc.tile_pool(name="work", bufs=3))
    psum_pool = ctx.enter_context(tc.tile_pool(name="psum", bufs=1, space="PSUM"))
    dram_pool = ctx.enter_context(tc.tile_pool(name="dram", bufs=1, space="DRAM"))

    # ---- constants ----
    iota_ql = const_pool.tile([128, QL], I32)
    nc.gpsimd.iota(iota_ql[:], pattern=[[1, QL]], base=0, channel_multiplier=0)
    iota_qh = const_pool.tile([128, QH], I32)
    nc.gpsimd.iota(iota_qh[:], pattern=[[1, QH]], base=0, channel_multiplier=0)
    ones_col = const_pool.tile([128, 1], F32)
    nc.vector.memset(ones_col[:], 1.0)
    ones_row = const_pool.tile([1, 128], F32)
    nc.vector.memset(ones_row[:], 1.0)

    a_sb = const_pool.tile([128, 6, NBINS], F32)
    nc.sync.dma_start(out=a_sb[:], in_=amat_in.ap())
    coefs = const_pool.tile([128, 12], F32)
    nc.sync.dma_start(out=coefs[:], in_=coefs_in.ap())
    mask_sb = const_pool.tile([128, chunks, cc], F32)
    nc.sync.dma_start(out=mask_sb[:], in_=mask_in.ap())

    # ---- xyz gather table: AllGather per-core frame slices, then broadcast
    # to [128, 4096] (row p = coord p%3). xyzg[f, c, a] = coord c of atom
    # f*512+a (rank-major concat of the [3, 512] contributions).
    # collectives may not read IO tensors: stage the slice into internal DRAM.
    # extract [3, 512] (partition c3, free a=hi*128+lo) from the blob columns.
    xyzs_sb = const_pool.tile([3, N_ATOMS], F32)
    nc.sync.dma_start(
        out=xyzs_sb[:].rearrange("c (h l) -> c h l", h=4),
        in_=xyz_words.rearrange("l (c h) -> c h l", c=3),
    )
    xyzs_loc = dram_pool.tile([3, N_ATOMS], F32)
    nc.sync.dma_start(out=xyzs_loc[:], in_=xyzs_sb[:])
    xyzg = dram_pool.tile([N_FRAMES, 3, N_ATOMS], F32)
    nc.gpsimd.collective_compute(
        "AllGather",
        OP.bypass,
        replica_groups=[list(range(N_CORES))],
        ins=[xyzs_loc[:].opt()],
        outs=[xyzg[:].opt()],
    )
    xyzt_loads = []
    for c3 in range(3):
        npart = len(range(c3, 128, 3))
        for f in range(N_FRAMES):
            bc = nc.sync.dma_start(
                out=xyzt_sb.ap()[c3:128:3, f * N_ATOMS : (f + 1) * N_ATOMS],
                in_=xyzg[f, c3 : c3 + 1, :].to_broadcast([npart, N_ATOMS]),
            )
            xyzt_loads.append(bc)

    prev_gather = {}  # chunk -> gather inst (ap_gather APs invisible to Tile)
    prev_repack = {}  # chunk -> [repack insts]

    psum_m = psum_pool.tile([QL * PMOM, QH], F32)  # [64, 12] moment accumulator

    def prep_chunk(k):
        alt = pool.tile([128, cc], I32, tag="alt")
        nc.sync.dma_start(out=alt[:], in_=al[:, k])

        # unpack packed word -> three flat-table indices t = f*512 + atom,
        # written as int16 into the ap_gather wrapped-index tile:
        # slot m = 3j + s (s minor).
        idxs16 = idxs16_raw[k % 2].ap()
        i16v = idxs16.rearrange("p (j s) -> p j s", s=3)
        f9 = pool.tile([128, cc], I32, tag="f9")
        nc.vector.tensor_scalar(out=f9[:], in0=alt[:], scalar1=7, scalar2=9,
                                op0=OP.bitwise_and, op1=OP.logical_shift_left)
        idx_copies = []
        for si, (sh, msk) in enumerate(((3, 511), (12, 511), (21, None))):
            tmp = pool.tile([128, cc], I32, tag=f"tmp{si}")
            if msk is not None:
                nc.vector.tensor_scalar(
                    out=tmp[:], in0=alt[:], scalar1=sh, scalar2=msk,
                    op0=OP.logical_shift_right, op1=OP.bitwise_and)
            else:
                nc.vector.tensor_scalar(
                    out=tmp[:], in0=alt[:], scalar1=sh, scalar2=None,
                    op0=OP.logical_shift_right)
            tv = pool.tile([128, cc], I32, tag=f"tv{si}")
            nc.vector.tensor_tensor(out=tv[:], in0=tmp[:], in1=f9[:],
                                    op=OP.bitwise_or)
            cp = nc.vector.tensor_copy(out=i16v[:, :, si], in_=tv[:])
            if k - 2 in prev_gather:  # WAR: slot reuse (2-deep raw buffers)
                add_dep_helper(cp.ins, prev_gather[k - 2].ins, reason="idxs16 WAR")
            idx_copies.append(cp)

        # GPSIMD gather: per 16-partition group g the idx stream unwraps as
        # n = m*16 + w (w = source partition%16, m = 3j+s); every partition p
        # of the group gathers the full stream from ITS table row (coord p%3)
        gath = gath_raw[k % 2].ap()
        # last chunk: only the first jlast columns hold real angles (p-minor
        # host order puts all pad at the tail); gather only those. The stale
        # tail of the gath buffer is finite and mask-zeroed downstream.
        ncols = cc
        if per is not None and k == chunks - 1:
            rem = per - (chunks - 1) * 128 * cc
            ncols = max(4, min(cc, -(-rem // 128)))
        gth = nc.gpsimd.ap_gather(
            out_ap=gath.unsqueeze(2),
            in_ap=xyzt_sb.ap().unsqueeze(2),
            idxs_ap=idxs16,
            channels=128,
            num_elems=N_FRAMES * N_ATOMS,
            d=1,
            num_idxs=3 * 16 * ncols,
        )
        for bc in xyzt_loads:
            add_dep_helper(gth.ins, bc.ins, reason="gather reads table")
        for cp in idx_copies:
            add_dep_helper(gth.ins, cp.ins, reason="gather reads idxs")
        if k - 2 in prev_repack:  # WAW on gath slot (2-deep raw buffers)
            for rp in prev_repack[k - 2]:
                add_dep_helper(gth.ins, rp.ins, reason="gath WAR vs old repack")
        prev_gather[k] = gth
        return gath, gth

    prepped = {0: prep_chunk(0)}
    for k in range(chunks):
        # issue next chunk's prep + gather BEFORE this chunk's math so the
        # Pool engine (bottleneck) is never starved by DVE trace order
        if k + 1 < chunks:
            prepped[k + 1] = prep_chunk(k + 1)
        gath, gth = prepped.pop(k)
        mask = mask_sb[:, k]

        # contiguous-block repack: math partition p' = 16g + w' takes stream
        # block n in [w'*3cc, (w'+1)*3cc) of its group from rep partition
        # 16g+c. Block = whole triplets since 3cc % 48 == 0. One contiguous
        # DMA per coordinate. In-block: n - w'*3cc = 48*jj + 16*s + w0, the
        # angle being (partition 16g+w0, col 4w'+jj).
        gc = []
        repacks = []
        # three engines: sync/scalar get their own Pool-sem waits; gpsimd
        # follows the gather in Pool program order. (A single engine would
        # leave repacks 2-3 wait-free and racing the gather across queues.)
        rp_engines = [nc.sync, nc.scalar, nc.sync]
        for c3 in range(3):
            gt = pool.tile([128, 3 * cc], F32, tag=f"gc{c3}")
            rp = rp_engines[c3].dma_start(out=gt[:], in_=gath[c3:128:16, :])
            add_dep_helper(rp.ins, gth.ins, reason="repack reads gather output")
            repacks.append(rp)
            gc.append(gt)
        prev_repack[k] = repacks

        # per-(coord, slot) views [128, jj(4), w0(16)] -> 64 angles/partition
        na = cc  # angles per partition per chunk (4*16)
        def sv(ci, si):
            return gc[ci][:].rearrange("p (j s w) -> p j s w", s=3, w=16)[:, :, si, :]

        d11 = pool.tile([128, na], F32, tag="d11")
        d22 = pool.tile([128, na], F32, tag="d22")
        d12 = pool.tile([128, na], F32, tag="d12")
        d11v = d11[:].rearrange("p (j w) -> p j w", w=16)
        d22v = d22[:].rearrange("p (j w) -> p j w", w=16)
        d12v = d12[:].rearrange("p (j w) -> p j w", w=16)
        v1c = pool.tile([128, cc // 16, 16], F32, tag="v1c")
        v2c = pool.tile([128, cc // 16, 16], F32, tag="v2c")
        mm = pool.tile([128, cc // 16, 16], F32, tag="mm")
        for ci in range(3):
            nc.vector.tensor_tensor(out=v1c[:], in0=sv(ci, 0), in1=sv(ci, 1), op=OP.subtract)
            nc.vector.tensor_tensor(out=v2c[:], in0=sv(ci, 2), in1=sv(ci, 1), op=OP.subtract)
            if ci == 0:
                nc.vector.tensor_tensor(out=d11v, in0=v1c[:], in1=v1c[:], op=OP.mult)
                nc.vector.tensor_tensor(out=d22v, in0=v2c[:], in1=v2c[:], op=OP.mult)
                nc.vector.tensor_tensor(out=d12v, in0=v1c[:], in1=v2c[:], op=OP.mult)
            else:
                nc.vector.tensor_tensor(out=mm[:], in0=v1c[:], in1=v1c[:], op=OP.mult)
                nc.vector.tensor_tensor(out=d11v, in0=d11v, in1=mm[:], op=OP.add)
                nc.vector.tensor_tensor(out=mm[:], in0=v2c[:], in1=v2c[:], op=OP.mult)
                nc.vector.tensor_tensor(out=d22v, in0=d22v, in1=mm[:], op=OP.add)
                nc.vector.tensor_tensor(out=mm[:], in0=v1c[:], in1=v2c[:], op=OP.mult)
                nc.vector.tensor_tensor(out=d12v, in0=d12v, in1=mm[:], op=OP.add)

        nn_ = pool.tile([128, cc], F32, tag="nn")
        nc.vector.tensor_tensor(out=nn_[:], in0=d11[:], in1=d22[:], op=OP.mult)
        sq = pool.tile([128, cc], F32, tag="sq")
        # bias keeps padded slots (zero vectors) finite: 1/sqrt(tiny) != inf*0
        nc.scalar.activation(sq[:], nn_[:], AF.Sqrt, bias=coefs[:, 8:9])
        rs = pool.tile([128, cc], F32, tag="rs")
        nc.vector.reciprocal(rs[:], sq[:])
        u = pool.tile([128, cc], F32, tag="u")
        nc.vector.tensor_tensor(out=u[:], in0=d12[:], in1=rs[:], op=OP.mult)
        # clamp |u| <= 1
        au0 = pool.tile([128, cc], F32, tag="au0")
        nc.scalar.activation(au0[:], u[:], AF.Abs)
        au = pool.tile([128, cc], F32, tag="au")
        nc.vector.tensor_scalar(
            out=au[:], in0=au0[:], scalar1=1.0, scalar2=None, op0=OP.min
        )
        sg = pool.tile([128, cc], F32, tag="sg")
        nc.scalar.activation(sg[:], u[:], AF.Sign)

        # theta_abs = sqrt(1-|u|) * P(|u|) in degrees (A&S 4.4.46, 8 terms);
        # theta = 90 + sg*(theta_abs - 90)
        sqterm = pool.tile([128, cc], F32, tag="sqterm")
        nc.scalar.activation(sqterm[:], au[:], AF.Sqrt, bias=1.0, scale=-1.0)
        x2 = pool.tile([128, cc], F32, tag="x2")
        nc.scalar.activation(x2[:], au[:], AF.Square)
        x4 = pool.tile([128, cc], F32, tag="x4")
        nc.scalar.activation(x4[:], x2[:], AF.Square)

        def pair(i_odd, col_even, tag):
            p = pool.tile([128, cc], F32, tag=tag)
            nc.vector.scalar_tensor_tensor(
                out=p[:], in0=au[:], scalar=float(ACOS_COEF[i_odd] * DEG),
                in1=coefs[:, col_even : col_even + 1].to_broadcast([128, cc]),
                op0=OP.mult, op1=OP.add,
            )
            return p

        p01 = pair(1, 0, "p01")
        p23 = pair(3, 2, "p23")
        p45 = pair(5, 4, "p45")
        p67 = pair(7, 6, "p67")
        t1 = pool.tile([128, cc], F32, tag="es1")
        nc.vector.tensor_tensor(out=t1[:], in0=x2[:], in1=p23[:], op=OP.mult)
        nc.vector.tensor_tensor(out=t1[:], in0=t1[:], in1=p01[:], op=OP.add)
        t2 = pool.tile([128, cc], F32, tag="es2")
        nc.vector.tensor_tensor(out=t2[:], in0=x2[:], in1=p67[:], op=OP.mult)
        nc.vector.tensor_tensor(out=t2[:], in0=t2[:], in1=p45[:], op=OP.add)
        nc.vector.tensor_tensor(out=t2[:], in0=t2[:], in1=x4[:], op=OP.mult)
        nc.vector.tensor_tensor(out=t1[:], in0=t1[:], in1=t2[:], op=OP.add)
        thabs = pool.tile([128, cc], F32, tag="thabs")
        nc.vector.tensor_tensor(out=thabs[:], in0=sqterm[:], in1=t1[:], op=OP.mult)
        theta = pool.tile([128, cc], F32, tag="theta")
        nc.vector.tensor_scalar(
            out=theta[:], in0=thabs[:], scalar1=-90.0, scalar2=None, op0=OP.add
        )
        nc.vector.tensor_tensor(out=theta[:], in0=theta[:], in1=sg[:], op=OP.mult)
        nc.vector.tensor_scalar(
            out=theta[:], in0=theta[:], scalar1=90.0, scalar2=None, op0=OP.add
        )

        # fine bin q = round(theta/H) (convert rounding handled by probe: trunc)
        qf_pre = pool.tile([128, cc], F32, tag="qfpre")
        nc.vector.tensor_scalar(
            out=qf_pre[:], in0=theta[:], scalar1=1.0 / H, scalar2=0.5,
            op0=OP.mult, op1=OP.add,
        )
        q_i = pool.tile([128, cc], I32, tag="qi")
        nc.vector.tensor_copy(out=q_i[:], in_=qf_pre[:])
        qf = pool.tile([128, cc], F32, tag="qf")
        nc.vector.tensor_copy(out=qf[:], in_=q_i[:])
        eps = pool.tile([128, cc], F32, tag="eps")
        nc.vector.scalar_tensor_tensor(
            out=eps[:], in0=qf[:], scalar=-H, in1=theta[:], op0=OP.mult, op1=OP.add
        )
        qh_i = pool.tile([128, cc], I32, tag="qhi")
        nc.vector.tensor_scalar(
            out=qh_i[:], in0=q_i[:], scalar1=int(math.log2(QL)), scalar2=None,
            op0=OP.arith_shift_right
        )
        ql_i = pool.tile([128, cc], I32, tag="qli")
        nc.vector.tensor_scalar(
            out=ql_i[:], in0=q_i[:], scalar1=QL - 1, scalar2=None, op0=OP.bitwise_and
        )

        # moment payload E = mask * (1, eps, eps^2, eps^3)
        ee = pool.tile([128, cc, PMOM], F32, tag="ee")
        nc.vector.tensor_copy(out=ee[:, :, 0], in_=mask)
        nc.vector.tensor_tensor(out=ee[:, :, 1], in0=eps[:], in1=mask, op=OP.mult)
        nc.vector.tensor_tensor(
            out=ee[:, :, 2], in0=ee[:, :, 1], in1=eps[:], op=OP.mult
        )
        nc.vector.tensor_tensor(
            out=ee[:, :, 3], in0=ee[:, :, 2], in1=eps[:], op=OP.mult
        )

        # one-hots
        oh_ql = pool.tile([128, cc, QL], F32, tag="ohql")
        nc.vector.tensor_tensor(
            out=oh_ql[:],
            in0=ql_i[:].unsqueeze(2).to_broadcast([128, cc, QL]),
            in1=iota_ql[:].unsqueeze(1).to_broadcast([128, cc, QL]),
            op=OP.is_equal,
        )
        oh_qh = pool.tile([128, cc, QH], F32, tag="ohqh")
        nc.vector.tensor_tensor(
            out=oh_qh[:],
            in0=qh_i[:].unsqueeze(2).to_broadcast([128, cc, QH]),
            in1=iota_qh[:].unsqueeze(1).to_broadcast([128, cc, QH]),
            op=OP.is_equal,
        )
        # lhsT[m, (ql, pm)] = oh_ql[m, ql] * E[m, pm]
        lhs = pool.tile([128, cc, QL * PMOM], F32, tag="lhs")
        nc.vector.tensor_tensor(
            out=lhs[:],
            in0=oh_ql[:].unsqueeze(3).to_broadcast([128, cc, QL, PMOM]),
            in1=ee[:].unsqueeze(2).to_broadcast([128, cc, QL, PMOM]),
            op=OP.mult,
        )

        for j in range(cc):
            nc.tensor.matmul(
                out=psum_m[:],
                lhsT=lhs[:, j, :],
                rhs=oh_qh[:, j, :],
                start=(k == 0 and j == 0),
                stop=(k == chunks - 1 and j == cc - 1),
            )

    # ---- allreduce moments ----
    m_sb = const_pool.tile([QL * PMOM, QH], F32)
    nc.vector.tensor_copy(out=m_sb[:], in_=psum_m[:])
    m_local = dram_pool.tile([QL * PMOM, QH], F32)
    nc.sync.dma_start(out=m_local[:], in_=m_sb[:])
    m_red = dram_pool.tile([QL * PMOM, QH], F32)
    nc.gpsimd.collective_compute(
        "AllReduce",
        OP.add,
        replica_groups=[list(range(N_CORES))],
        ins=[m_local[:].opt()],
        outs=[m_red[:].opt()],
    )
    # reload flat: element kk = p*QH + n ; rhs chunks [128, 6]
    m_rhs = const_pool.tile([128, 6], F32)
    nc.sync.dma_start(
        out=m_rhs[:], in_=m_red[:].rearrange("p n -> (p n)").rearrange("(c p) -> p c", p=128)
    )

    # ---- final contraction count[b] = sum_k M[k] * A[k, b] ----
    psum_ca = psum_pool.tile([128, 1], F32)
    psum_cb = psum_pool.tile([NBINS - 128, 1], F32)
    for cquad in range(6):
        nc.tensor.matmul(
            out=psum_ca[:], lhsT=a_sb[:, cquad, 0:128], rhs=m_rhs[:, cquad : cquad + 1],
            start=(cquad == 0), stop=(cquad == 5),
        )
    for cquad in range(6):
        nc.tensor.matmul(
            out=psum_cb[:], lhsT=a_sb[:, cquad, 128:NBINS], rhs=m_rhs[:, cquad : cquad + 1],
            start=(cquad == 0), stop=(cquad == 5),
        )
    cnt = const_pool.tile([128, 2], F32)
    nc.vector.memset(cnt[:], 0.0)
    nc.vector.tensor_copy(out=cnt[:, 0:1], in_=psum_ca[:])
    nc.vector.tensor_copy(out=cnt[0 : NBINS - 128, 1:2], in_=psum_cb[:])

    # total + normalize
    psum_t = psum_pool.tile([1, 2], F32)
    nc.tensor.matmul(out=psum_t[:], lhsT=ones_col[:], rhs=cnt[:], start=True, stop=True)
    tt = const_pool.tile([1, 2], F32)
    nc.vector.tensor_copy(out=tt[:], in_=psum_t[:])
    tot = const_pool.tile([1, 1], F32)
    nc.vector.tensor_tensor(out=tot[:], in0=tt[:, 0:1], in1=tt[:, 1:2], op=OP.add)
    rtot = const_pool.tile([1, 1], F32)
    nc.vector.reciprocal(rtot[:], tot[:])
    psum_r = psum_pool.tile([128, 1], F32)
    nc.tensor.matmul(out=psum_r[:], lhsT=ones_row[:], rhs=rtot[:], start=True, stop=True)
    outn = const_pool.tile([128, 2], F32)
    nc.vector.tensor_tensor(
        out=outn[:], in0=cnt[:], in1=psum_r[:].to_broadcast([128, 2]), op=OP.mult
    )
    nc.sync.dma_start(out=out[0:128], in_=outn[:, 0])
    nc.sync.dma_start(out=out[128:NBINS], in_=outn[0 : NBINS - 128, 1])


# ---------------- host side ----------------

def pack_inputs(xyz: np.ndarray, angle_list: np.ndarray, per: int):
    """Pack host-side into ONE per-core blob: packed int32 angle words plus
    this core's xyz frame slice (f32 bits in the last 12 columns).

    Returns blob_global [N_CORES*128, CHUNKS*C+12] int32."""
    a = np.asarray(angle_list)
    if a.dtype == np.int64 and a.flags.c_contiguous:
        a = a.view(np.int32)[:, 0::2]  # little-endian low words, no copy
    elif a.dtype != np.int32:
        a = a.astype(np.int32)
    w = a[:, 3] << 21  # [M] packed word (f | i<<3 | c<<12 | j<<21)
    w |= a[:, 2] << 12
    w |= a[:, 1] << 3
    w |= a[:, 0]
    n_cores = N_CORES
    blob = np.zeros((n_cores, 128, CHUNKS * C + 12), dtype=np.int32)
    wp = np.zeros((n_cores, SLOTS), dtype=np.int32)
    wp[:, :per] = w.reshape(n_cores, per)
    # p-minor slot order: s' = (k*C + j)*128 + p  ->  [128, CHUNKS*C]
    blob[:, :, : CHUNKS * C] = wp.reshape(n_cores, CHUNKS * C, 128).transpose(0, 2, 1)

    # xyz word (c3, a=hi*128+lo) of core r (= frame r) -> blob[r, lo, 512+c3*4+hi]
    xyzs = np.ascontiguousarray(
        np.asarray(xyz, dtype=np.float32).transpose(0, 2, 1)
    ).view(np.int32)  # [8, 3, 512]
    xw = xyzs.reshape(n_cores, 3, 4, 128).transpose(0, 3, 1, 2)  # [r, lo, c3, hi]
    blob[:, :, CHUNKS * C :] = xw.reshape(n_cores, 128, 12)
    return blob.reshape(n_cores * 128, CHUNKS * C + 12)


_PROG_CACHE = {}


def build_program(per=PER_CORE, chunks=CHUNKS, cols=C):
    key = (per, chunks, cols)
    if key in _PROG_CACHE:
        return _PROG_CACHE[key]
    nc = bacc.Bacc("TRN2", target_bir_lowering=False, num_devices=N_CORES)
    ins = {
        "blob": nc.dram_tensor(
            "blob", [128, chunks * cols + 12], I32, kind="ExternalInput"
        ).ap(),
    }
    consts = (
        nc.inline_tensor(build_mask_math(per, chunks, cols), name="maskc"),
        nc.inline_tensor(amat_inline(), name="amatc"),
        nc.inline_tensor(coefs_tile(), name="coefsc"),
    )
    outs = {"out": nc.dram_tensor("out", [NBINS], F32, kind="ExternalOutput").ap()}
    # raw ap_gather buffers: must be allocated BEFORE TileContext so the tile
    # pools (which claim the free SBUF region at entry) don't overlap them.
    xyzt_sb = nc.alloc_sbuf_tensor("xyzt_sb", [128, N_FRAMES * N_ATOMS], F32)
    idxs16_raw = [
        nc.alloc_sbuf_tensor(f"idxs16r{i}", [128, 3 * cols], mybir.dt.int16)
        for i in range(2)
    ]
    gath_raw = [
        nc.alloc_sbuf_tensor(f"gathr{i}", [128, 3 * 16 * cols], F32)
        for i in range(2)
    ]
    raw = (xyzt_sb, idxs16_raw, gath_raw)
    with tile.TileContext(nc) as tc:
        adf_kernel(tc, outs, ins, raw, consts, per=per)
    nc.compile()
    _PROG_CACHE[key] = nc
    return nc


_RUNNER_CACHE = {}


def _get_runner(per=PER_CORE):
    """Build (once) the cached jitted SPMD executor for the bass program."""
    key = per
    if key in _RUNNER_CACHE:
        return _RUNNER_CACHE[key]

    import jax
    from jax.sharding import Mesh, PartitionSpec
    from jax.experimental.shard_map import shard_map
    from concourse import bass2jax

    nc = build_program(per=per)
    bass2jax.install_neuronx_cc_hook()

    partition_name = nc.partition_id_tensor.name if nc.partition_id_tensor else None
    in_names, out_names, out_avals = [], [], []
    for alloc in nc.m.functions[0].allocations:
        if not isinstance(alloc, mybir.MemoryLocationSet):
            continue
        name = alloc.memorylocations[0].name
        if alloc.kind == "ExternalInput":
            if name != partition_name:
                in_names.append(name)
        elif alloc.kind == "ExternalOutput":
            out_names.append(name)
            out_avals.append(
                jax.core.ShapedArray(tuple(alloc.tensor_shape), mybir.dt.np(alloc.dtype))
            )
    assert in_names == ["blob"], in_names
    assert out_names == ["out"], out_names
    all_in = list(in_names) + ([partition_name] if partition_name else [])

    def _body(*args):
        operands = list(args)
        if partition_name:
            operands.append(bass2jax.partition_id_tensor())
        return tuple(
            bass2jax._bass_exec_p.bind(
                *operands,
                out_avals=tuple(out_avals),
                in_names=tuple(all_in),
                out_names=tuple(out_names),
                lowering_input_output_aliases=(),
                sim_require_finite=True,
                sim_require_nnan=True,
                nc=nc,
            )
        )

    devices = jax.devices()[:N_CORES]
    assert len(devices) == N_CORES
    mesh = Mesh(np.asarray(devices), ("core",))
    fn = jax.jit(
        shard_map(
            _body,
            mesh=mesh,
            in_specs=tuple(PartitionSpec("core") for _ in in_names),
            out_specs=tuple(PartitionSpec("core") for _ in out_names),
            check_rep=False,
        )
    )
    runner = (fn, out_avals)
    _RUNNER_CACHE[key] = runner
    return runner


def kernel(**inputs) -> np.ndarray:
    import time as _time

    xyz = np.asarray(inputs["xyz"], dtype=np.float32)
    angle_list = np.asarray(inputs["angle_list"])
    m = angle_list.shape[0]
    assert m % N_CORES == 0, f"angle count {m} must divide across {N_CORES} cores"
    per = m // N_CORES
    assert per <= SLOTS, f"angle count {m} exceeds kernel capacity"

    fn, out_avals = _get_runner(per=per)
    blob_g = pack_inputs(xyz, angle_list, per)

    import jax
    from jax.sharding import Mesh, NamedSharding, PartitionSpec

    mesh = Mesh(np.asarray(jax.devices()[:N_CORES]), ("core",))
    shard = NamedSharding(mesh, PartitionSpec("core"))

    t0 = _time.time()
    # async put: the transfer starts immediately and overlaps with dispatch
    d_blob = jax.device_put(blob_g, shard)
    outs = fn(d_blob)
    # all cores hold the identical allreduced histogram; fetch core 0's shard
    out0 = np.asarray(outs[0].addressable_shards[0].data)
    kernel._last_run_s = _time.time() - t0
    kernel._last_results = None
    return np.asarray(out0, dtype=np.float32).reshape(NBINS)


if __name__ == "__main__":
    # smoke: build only
    build_program()
    print("program built ok")


# revision 22
# speedup vs baseline: 1.1036x; 1.1036x over previous
"""Trainium2 Bass kernel for nn_DifferentiableADF (angular distribution function).

Computes: for M=500k angle triplets over xyz[8,512,3], the Gaussian-smeared
180-bin histogram of bond angles, normalized to sum 1.

Strategy (8 cores, data-parallel over angles):
  - angle_list sharded M/8 per core, each angle packed host-side into ONE
    int32 word (f | i<<3 | c<<12 | j<<21); unpacked on the DVE into three
    int16 flat-table indices t = f*512 + atom. This is the only large
    per-call transfer (256 KiB/core) -- the axon tunnel is the bottleneck
    (~90 ms fixed RPC + ~87 MB/s), so input bytes are minimized.
  - xyz ships sharded: core r carries only frame r's atoms as [3, 512]
    (6 KiB); an on-device AllGather assembles the full flat table, then
    0-stride DMAs broadcast it to the [128, 4096] gather table (row p =
    coord p%3).
  - mask / smearing matrix / coefficients are inline_tensor Consts baked
    into the NEFF (loaded to HBM once at model load, zero per-call cost).
  - per chunk: GPSIMD ap_gather fetches coords; a contiguous-block DMA
    repack aligns the stream to compute partitions. Bond vectors + dots on
    DVE, arccos via A&S 4.4.46 polynomial, fast-Gauss-transform moment
    accumulation: theta -> nearest fine bin q, moments (1, eps, eps^2,
    eps^3) scattered into bins via a digit-split one-hot matmul on the PE
    (PSUM accumulates across all chunks).
  - AllReduce of the [64,12] moment block, then a tiny matmul against a
    precomputed Hermite-derivative matrix reconstructs the exact smeared
    histogram; normalized on device. All cores produce identical output.
  - Execution: a cached jax.jit(shard_map) over the bass_exec custom call
    (built once per process); per-call cost is one execute RPC with the
    packed inputs pipelined in.
"""

import math
import os
import sys
from contextlib import ExitStack

import numpy as np

sys.path.insert(0, "/opt/trn_rl_repo")

import concourse.bass as bass  # noqa: E402
import concourse.tile as tile  # noqa: E402
from concourse.tile import add_dep_helper  # noqa: E402
from concourse import bacc, mybir  # noqa: E402
from concourse._compat import with_exitstack  # noqa: E402

F32 = mybir.dt.float32
I32 = mybir.dt.int32
I16 = mybir.dt.int16
AF = mybir.ActivationFunctionType
OP = mybir.AluOpType

# ---------------- problem constants ----------------
N_FRAMES = 8
N_ATOMS = 512
N_ANGLES = 500_000
NBINS = 180
H = 180.0 / 179.0  # bin spacing == fine-grid spacing
N_CORES = 8
PER_CORE = N_ANGLES // N_CORES  # 62500

QL = 8   # low digit of fine-bin index
QH = 24  # high digit (8*24 = 192 >= 180 bins; q in [0,191] all valid rows)
PMOM = 4  # moments kept: eps^0..eps^3
KFLAT = QL * PMOM * QH  # 768 = 6*128
DEG = 180.0 / math.pi

# layout: angle slot s = ((p*CHUNKS + k)*C + j)  p: partition, k: chunk, j: col
CHUNKS = 8
C = 64  # must be multiple of 16 (contiguous-block repack needs 3C % 48 == 0)
SLOTS = 128 * CHUNKS * C  # 65536 >= 62500

# Abramowitz & Stegun 4.4.46: arccos(x) = sqrt(1-x) * sum a_k x^k, x in [0,1]
ACOS_COEF = [
    1.5707963050, -0.2145988016, 0.0889789874, -0.0501743046,
    0.0308918810, -0.0170881256, 0.0066700901, -0.0012624911,
]


def build_amat() -> np.ndarray:
    """A[(ql*PMOM+pm)*QH+qh, b] = g^(pm)(c_q - o_b)/pm!  with g = exp(-x^2/2)."""
    q = np.arange(QL * QH, dtype=np.float64)
    b = np.arange(NBINS, dtype=np.float64)
    d = q[:, None] * H - b[None, :] * H  # [192, 180]
    g0 = np.exp(-0.5 * d * d)
    derivs = [g0, -d * g0, (d * d - 1.0) / 2.0 * g0, (3.0 * d - d**3) / 6.0 * g0]
    a = np.zeros((KFLAT, NBINS), dtype=np.float64)
    for qi in range(QL * QH):
        ql, qh = qi % QL, qi // QL
        for pm in range(PMOM):
            a[(ql * PMOM + pm) * QH + qh, :] = derivs[pm][qi, :]
    return a.astype(np.float32)


def build_mask_math(per: int, chunks: int, cols: int) -> np.ndarray:
    """Validity mask in the post-repack math layout (p-minor slot order).

    original slot s' = (k*cols + j)*128 + p is valid iff s' < per; math slot
    (p'=16g+w', k, 16*jj + w0) maps to (p=16g+w0, k, j=(cols//16)*w' + jj)."""
    jj = cols // 16
    k_, j_, p_ = np.meshgrid(
        np.arange(chunks), np.arange(cols), np.arange(128), indexing="ij"
    )
    valid = ((k_ * cols + j_) * 128 + p_ < per).astype(np.float32)  # [k, j, p]
    v = valid.transpose(2, 0, 1)  # [p, k, j]
    m = v.reshape(8, 16, chunks, 16, jj)  # [g, w0, k, w', jj]
    mm = np.transpose(m, (0, 3, 2, 4, 1))  # [g, w', k, jj, w0]
    return np.ascontiguousarray(mm.reshape(128, chunks, cols))


def coefs_tile() -> np.ndarray:
    row = np.zeros(12, dtype=np.float32)
    row[:8] = np.array([c * DEG for c in ACOS_COEF], dtype=np.float32)
    row[8] = 1e-30
    return np.broadcast_to(row, (128, 12)).copy()


def amat_inline() -> np.ndarray:
    """amat pre-rearranged to the SBUF layout [128, 6, NBINS]:
    element (p, cq, b) = amat[cq*128 + p, b]."""
    a = build_amat()  # [768, 180]
    return np.ascontiguousarray(a.reshape(6, 128, NBINS).transpose(1, 0, 2))


@with_exitstack
def adf_kernel(ctx: ExitStack, tc: tile.TileContext, outs, ins, raw, consts,
               per=None):
    nc = tc.nc
    xyzt_sb, idxs16_raw, gath_raw = raw
    blob = ins["blob"]      # [128, CHUNKS*C+12] int32: packed angles + xyz bits
    nbl = blob.shape[1]
    al = blob[:, 0 : nbl - 12].rearrange(
        "p (k c) -> p k c", k=CHUNKS
    )                       # [128, CHUNKS, C] int32 packed (f|i<<3|c<<12|j<<21)
    # xyz slice (f32 bits): word (c3, a=hi*128+lo) is at blob[lo, nbl-12+c3*4+hi]
    xyz_words = blob[:, nbl - 12 : nbl].bitcast(F32)
    mask_in, amat_in, coefs_in = consts  # inline Const DRAM handles
    out = outs["out"]       # [180] f32

    chunks, cc = al.shape[1], al.shape[2]

    const_pool = ctx.enter_context(tc.tile_pool(name="const", bufs=1))
    pool = ctx.enter_context(tc.tile_pool(name="work", bufs=3))
    psum_pool = ctx.enter_context(tc.tile_pool(name="psum", bufs=1, space="PSUM"))
    dram_pool = ctx.enter_context(tc.tile_pool(name="dram", bufs=1, space="DRAM"))

    # ---- constants ----
    iota_ql = const_pool.tile([128, QL], I32)
    nc.gpsimd.iota(iota_ql[:], pattern=[[1, QL]], base=0, channel_multiplier=0)
    iota_qh = const_pool.tile([128, QH], I32)
    nc.gpsimd.iota(iota_qh[:], pattern=[[1, QH]], base=0, channel_multiplier=0)
    ones_col = const_pool.tile([128, 1], F32)
    nc.vector.memset(ones_col[:], 1.0)
    ones_row = const_pool.tile([1, 128], F32)
    nc.vector.memset(ones_row[:], 1.0)

    a_sb = const_pool.tile([128, 6, NBINS], F32)
    nc.sync.dma_start(out=a_sb[:], in_=amat_in.ap())
    coefs = const_pool.tile([128, 12], F32)
    nc.sync.dma_start(out=coefs[:], in_=coefs_in.ap())
    mask_sb = const_pool.tile([128, chunks, cc], F32)
    nc.sync.dma_start(out=mask_sb[:], in_=mask_in.ap())

    # ---- xyz gather table: AllGather per-core frame slices, then broadcast
    # to [128, 4096] (row p = coord p%3). xyzg[f, c, a] = coord c of atom
    # f*512+a (rank-major concat of the [3, 512] contributions).
    # collectives may not read IO tensors: stage the slice into internal DRAM.
    # extract [3, 512] (partition c3, free a=hi*128+lo) from the blob columns.
    xyzs_sb = const_pool.tile([3, N_ATOMS], F32)
    nc.sync.dma_start(
        out=xyzs_sb[:].rearrange("c (h l) -> c h l", h=4),
        in_=xyz_words.rearrange("l (c h) -> c h l", c=3),
    )
    xyzs_loc = dram_pool.tile([3, N_ATOMS], F32)
    nc.sync.dma_start(out=xyzs_loc[:], in_=xyzs_sb[:])
    xyzg = dram_pool.tile([N_FRAMES, 3, N_ATOMS], F32)
    nc.gpsimd.collective_compute(
        "AllGather",
        OP.bypass,
        replica_groups=[list(range(N_CORES))],
        ins=[xyzs_loc[:].opt()],
        outs=[xyzg[:].opt()],
    )
    xyzt_loads = []
    for c3 in range(3):
        npart = len(range(c3, 128, 3))
        for f in range(N_FRAMES):
            bc = nc.sync.dma_start(
                out=xyzt_sb.ap()[c3:128:3, f * N_ATOMS : (f + 1) * N_ATOMS],
                in_=xyzg[f, c3 : c3 + 1, :].to_broadcast([npart, N_ATOMS]),
            )
            xyzt_loads.append(bc)

    prev_gather = {}  # chunk -> gather inst (ap_gather APs invisible to Tile)
    prev_repack = {}  # chunk -> [repack insts]

    psum_m = psum_pool.tile([QL * PMOM, QH], F32)  # [64, 12] moment accumulator

    def prep_chunk(k):
        alt = pool.tile([128, cc], I32, tag="alt")
        nc.sync.dma_start(out=alt[:], in_=al[:, k])

        # unpack packed word -> three flat-table indices t = f*512 + atom,
        # written as int16 into the ap_gather wrapped-index tile:
        # slot m = 3j + s (s minor).
        idxs16 = idxs16_raw[k % 2].ap()
        i16v = idxs16.rearrange("p (j s) -> p j s", s=3)
        f9 = pool.tile([128, cc], I32, tag="f9")
        nc.vector.tensor_scalar(out=f9[:], in0=alt[:], scalar1=7, scalar2=9,
                                op0=OP.bitwise_and, op1=OP.logical_shift_left)
        idx_copies = []
        for si, (sh, msk) in enumerate(((3, 511), (12, 511), (21, None))):
            tmp = pool.tile([128, cc], I32, tag=f"tmp{si}")
            if msk is not None:
                nc.vector.tensor_scalar(
                    out=tmp[:], in0=alt[:], scalar1=sh, scalar2=msk,
                    op0=OP.logical_shift_right, op1=OP.bitwise_and)
            else:
                nc.vector.tensor_scalar(
                    out=tmp[:], in0=alt[:], scalar1=sh, scalar2=None,
                    op0=OP.logical_shift_right)
            tv = pool.tile([128, cc], I32, tag=f"tv{si}")
            nc.vector.tensor_tensor(out=tv[:], in0=tmp[:], in1=f9[:],
                                    op=OP.bitwise_or)
            cp = nc.vector.tensor_copy(out=i16v[:, :, si], in_=tv[:])
            if k - 2 in prev_gather:  # WAR: slot reuse (2-deep raw buffers)
                add_dep_helper(cp.ins, prev_gather[k - 2].ins, reason="idxs16 WAR")
            idx_copies.append(cp)

        # GPSIMD gather: per 16-partition group g the idx stream unwraps as
        # n = m*16 + w (w = source partition%16, m = 3j+s); every partition p
        # of the group gathers the full stream from ITS table row (coord p%3)
        gath = gath_raw[k % 2].ap()
        # last chunk: only the first jlast columns hold real angles (p-minor
        # host order puts all pad at the tail); gather only those. The stale
        # tail of the gath buffer is finite and mask-zeroed downstream.
        ncols = cc
        if per is not None and k == chunks - 1:
            rem = per - (chunks - 1) * 128 * cc
            ncols = max(4, min(cc, -(-rem // 128)))
        gth = nc.gpsimd.ap_gather(
            out_ap=gath.unsqueeze(2),
            in_ap=xyzt_sb.ap().unsqueeze(2),
            idxs_ap=idxs16,
            channels=128,
            num_elems=N_FRAMES * N_ATOMS,
            d=1,
            num_idxs=3 * 16 * ncols,
        )
        for bc in xyzt_loads:
            add_dep_helper(gth.ins, bc.ins, reason="gather reads table")
        for cp in idx_copies:
            add_dep_helper(gth.ins, cp.ins, reason="gather reads idxs")
        if k - 2 in prev_repack:  # WAW on gath slot (2-deep raw buffers)
            for rp in prev_repack[k - 2]:
                add_dep_helper(gth.ins, rp.ins, reason="gath WAR vs old repack")
        prev_gather[k] = gth
        return gath, gth

    prepped = {0: prep_chunk(0)}
    for k in range(chunks):
        # issue next chunk's prep + gather BEFORE this chunk's math so the
        # Pool engine (bottleneck) is never starved by DVE trace order
        if k + 1 < chunks:
            prepped[k + 1] = prep_chunk(k + 1)
        gath, gth = prepped.pop(k)
        mask = mask_sb[:, k]

        # contiguous-block repack: math partition p' = 16g + w' takes stream
        # block n in [w'*3cc, (w'+1)*3cc) of its group from rep partition
        # 16g+c. Block = whole triplets since 3cc % 48 == 0. One contiguous
        # DMA per coordinate. In-block: n - w'*3cc = 48*jj + 16*s + w0, the
        # angle being (partition 16g+w0, col 4w'+jj).
        gc = []
        repacks = []
        # three engines: sync/scalar get their own Pool-sem waits; gpsimd
        # follows the gather in Pool program order. (A single engine would
        # leave repacks 2-3 wait-free and racing the gather across queues.)
        rp_engines = [nc.sync, nc.scalar, nc.sync]
        for c3 in range(3):
            gt = pool.tile([128, 3 * cc], F32, tag=f"gc{c3}")
            rp = rp_engines[c3].dma_start(out=gt[:], in_=gath[c3:128:16, :])
            add_dep_helper(rp.ins, gth.ins, reason="repack reads gather output")
            repacks.append(rp)
            gc.append(gt)
        prev_repack[k] = repacks

        # per-(coord, slot) views [128, jj(4), w0(16)] -> 64 angles/partition
        na = cc  # angles per partition per chunk (4*16)
        def sv(ci, si):
            return gc[ci][:].rearrange("p (j s w) -> p j s w", s=3, w=16)[:, :, si, :]

        d11 = pool.tile([128, na], F32, tag="d11")
        d22 = pool.tile([128, na], F32, tag="d22")
        d12 = pool.tile([128, na], F32, tag="d12")
        d11v = d11[:].rearrange("p (j w) -> p j w", w=16)
        d22v = d22[:].rearrange("p (j w) -> p j w", w=16)
        d12v = d12[:].rearrange("p (j w) -> p j w", w=16)
        v1c = pool.tile([128, cc // 16, 16], F32, tag="v1c")
        v2c = pool.tile([128, cc // 16, 16], F32, tag="v2c")
        mm = pool.tile([128, cc // 16, 16], F32, tag="mm")
        for ci in range(3):
            nc.vector.tensor_tensor(out=v1c[:], in0=sv(ci, 0), in1=sv(ci, 1), op=OP.subtract)
            nc.vector.tensor_tensor(out=v2c[:], in0=sv(ci, 2), in1=sv(ci, 1), op=OP.subtract)
            if ci == 0:
                nc.vector.tensor_tensor(out=d11v, in0=v1c[:], in1=v1c[:], op=OP.mult)
                nc.vector.tensor_tensor(out=d22v, in0=v2c[:], in1=v2c[:], op=OP.mult)
                nc.vector.tensor_tensor(out=d12v, in0=v1c[:], in1=v2c[:], op=OP.mult)
            else:
                nc.vector.tensor_tensor(out=mm[:], in0=v1c[:], in1=v1c[:], op=OP.mult)
                nc.vector.tensor_tensor(out=d11v, in0=d11v, in1=mm[:], op=OP.add)
                nc.vector.tensor_tensor(out=mm[:], in0=v2c[:], in1=v2c[:], op=OP.mult)
                nc.vector.tensor_tensor(out=d22v, in0=d22v, in1=mm[:], op=OP.add)
                nc.vector.tensor_tensor(out=mm[:], in0=v1c[:], in1=v2c[:], op=OP.mult)
                nc.vector.tensor_tensor(out=d12v, in0=d12v, in1=mm[:], op=OP.add)

        nn_ = pool.tile([128, cc], F32, tag="nn")
        nc.vector.tensor_tensor(out=nn_[:], in0=d11[:], in1=d22[:], op=OP.mult)
        sq = pool.tile([128, cc], F32, tag="sq")
        # bias keeps padded slots (zero vectors) finite: 1/sqrt(tiny) != inf*0
        nc.scalar.activation(sq[:], nn_[:], AF.Sqrt, bias=coefs[:, 8:9])
        rs = pool.tile([128, cc], F32, tag="rs")
        nc.vector.reciprocal(rs[:], sq[:])
        u = pool.tile([128, cc], F32, tag="u")
        nc.vector.tensor_tensor(out=u[:], in0=d12[:], in1=rs[:], op=OP.mult)
        # clamp |u| <= 1
        au0 = pool.tile([128, cc], F32, tag="au0")
        nc.scalar.activation(au0[:], u[:], AF.Abs)
        au = pool.tile([128, cc], F32, tag="au")
        nc.vector.tensor_scalar(
            out=au[:], in0=au0[:], scalar1=1.0, scalar2=None, op0=OP.min
        )
        sg = pool.tile([128, cc], F32, tag="sg")
        nc.scalar.activation(sg[:], u[:], AF.Sign)

        # theta_abs = sqrt(1-|u|) * P(|u|) in degrees (A&S 4.4.46, 8 terms);
        # theta = 90 + sg*(theta_abs - 90)
        sqterm = pool.tile([128, cc], F32, tag="sqterm")
        nc.scalar.activation(sqterm[:], au[:], AF.Sqrt, bias=1.0, scale=-1.0)
        x2 = pool.tile([128, cc], F32, tag="x2")
        nc.scalar.activation(x2[:], au[:], AF.Square)
        x4 = pool.tile([128, cc], F32, tag="x4")
        nc.scalar.activation(x4[:], x2[:], AF.Square)

        def pair(i_odd, col_even, tag):
            p = pool.tile([128, cc], F32, tag=tag)
            nc.vector.scalar_tensor_tensor(
                out=p[:], in0=au[:], scalar=float(ACOS_COEF[i_odd] * DEG),
                in1=coefs[:, col_even : col_even + 1].to_broadcast([128, cc]),
                op0=OP.mult, op1=OP.add,
            )
            return p

        p01 = pair(1, 0, "p01")
        p23 = pair(3, 2, "p23")
        p45 = pair(5, 4, "p45")
        p67 = pair(7, 6, "p67")
        t1 = pool.tile([128, cc], F32, tag="es1")
        nc.vector.tensor_tensor(out=t1[:], in0=x2[:], in1=p23[:], op=OP.mult)
        nc.vector.tensor_tensor(out=t1[:], in0=t1[:], in1=p01[:], op=OP.add)
        t2 = pool.tile([128, cc], F32, tag="es2")
        nc.vector.tensor_tensor(out=t2[:], in0=x2[:], in1=p67[:], op=OP.mult)
        nc.vector.tensor_tensor(out=t2[:], in0=t2[:], in1=p45[:], op=OP.add)
        nc.vector.tensor_tensor(out=t2[:], in0=t2[:], in1=x4[:], op=OP.mult)
        nc.vector.tensor_tensor(out=t1[:], in0=t1[:], in1=t2[:], op=OP.add)
        thabs = pool.tile([128, cc], F32, tag="thabs")
        nc.vector.tensor_tensor(out=thabs[:], in0=sqterm[:], in1=t1[:], op=OP.mult)
        theta = pool.tile([128, cc], F32, tag="theta")
        nc.vector.tensor_scalar(
            out=theta[:], in0=thabs[:], scalar1=-90.0, scalar2=None, op0=OP.add
        )
        nc.vector.tensor_tensor(out=theta[:], in0=theta[:], in1=sg[:], op=OP.mult)
        nc.vector.tensor_scalar(
            out=theta[:], in0=theta[:], scalar1=90.0, scalar2=None, op0=OP.add
        )

        # fine bin q = round(theta/H) (convert rounding handled by probe: trunc)
        qf_pre = pool.tile([128, cc], F32, tag="qfpre")
        nc.vector.tensor_scalar(
            out=qf_pre[:], in0=theta[:], scalar1=1.0 / H, scalar2=0.5,
            op0=OP.mult, op1=OP.add,
        )
        q_i = pool.tile([128, cc], I32, tag="qi")
        nc.vector.tensor_copy(out=q_i[:], in_=qf_pre[:])
        qf = pool.tile([128, cc], F32, tag="qf")
        nc.vector.tensor_copy(out=qf[:], in_=q_i[:])
        eps = pool.tile([128, cc], F32, tag="eps")
        nc.vector.scalar_tensor_tensor(
            out=eps[:], in0=qf[:], scalar=-H, in1=theta[:], op0=OP.mult, op1=OP.add
        )
        qh_i = pool.tile([128, cc], I32, tag="qhi")
        nc.vector.tensor_scalar(
            out=qh_i[:], in0=q_i[:], scalar1=int(math.log2(QL)), scalar2=None,
            op0=OP.arith_shift_right
        )
        ql_i = pool.tile([128, cc], I32, tag="qli")
        nc.vector.tensor_scalar(
            out=ql_i[:], in0=q_i[:], scalar1=QL - 1, scalar2=None, op0=OP.bitwise_and
        )

        # moment payload E = mask * (1, eps, eps^2, eps^3)
        ee = pool.tile([128, cc, PMOM], F32, tag="ee")
        nc.vector.tensor_copy(out=ee[:, :, 0], in_=mask)
        nc.vector.tensor_tensor(out=ee[:, :, 1], in0=eps[:], in1=mask, op=OP.mult)
        nc.vector.tensor_tensor(
            out=ee[:, :, 2], in0=ee[:, :, 1], in1=eps[:], op=OP.mult
        )
        nc.vector.tensor_tensor(
            out=ee[:, :, 3], in0=ee[:, :, 2], in1=eps[:], op=OP.mult
        )

        # one-hots
        oh_ql = pool.tile([128, cc, QL], F32, tag="ohql")
        nc.vector.tensor_tensor(
            out=oh_ql[:],
            in0=ql_i[:].unsqueeze(2).to_broadcast([128, cc, QL]),
            in1=iota_ql[:].unsqueeze(1).to_broadcast([128, cc, QL]),
            op=OP.is_equal,
        )
        oh_qh = pool.tile([128, cc, QH], F32, tag="ohqh")
        nc.vector.tensor_tensor(
            out=oh_qh[:],
            in0=qh_i[:].unsqueeze(2).to_broadcast([128, cc, QH]),
            in1=iota_qh[:].unsqueeze(1).to_broadcast([128, cc, QH]),
            op=OP.is_equal,
        )
        # lhsT[m, (ql, pm)] = oh_ql[m, ql] * E[m, pm]
        lhs = pool.tile([128, cc, QL * PMOM], F32, tag="lhs")
        nc.vector.tensor_tensor(
            out=lhs[:],
            in0=oh_ql[:].unsqueeze(3).to_broadcast([128, cc, QL, PMOM]),
            in1=ee[:].unsqueeze(2).to_broadcast([128, cc, QL, PMOM]),
            op=OP.mult,
        )

        for j in range(cc):
            nc.tensor.matmul(
                out=psum_m[:],
                lhsT=lhs[:, j, :],
                rhs=oh_qh[:, j, :],
                start=(k == 0 and j == 0),
                stop=(k == chunks - 1 and j == cc - 1),
            )

    # ---- allreduce moments ----
    m_sb = const_pool.tile([QL * PMOM, QH], F32)
    nc.vector.tensor_copy(out=m_sb[:], in_=psum_m[:])
    m_local = dram_pool.tile([QL * PMOM, QH], F32)
    nc.sync.dma_start(out=m_local[:], in_=m_sb[:])
    m_red = dram_pool.tile([QL * PMOM, QH], F32)
    nc.gpsimd.collective_compute(
        "AllReduce",
        OP.add,
        replica_groups=[list(range(N_CORES))],
        ins=[m_local[:].opt()],
        outs=[m_red[:].opt()],
    )
    # reload flat: element kk = p*QH + n ; rhs chunks [128, 6]
    m_rhs = const_pool.tile([128, 6], F32)
    nc.sync.dma_start(
        out=m_rhs[:], in_=m_red[:].rearrange("p n -> (p n)").rearrange("(c p) -> p c", p=128)
    )

    # ---- final contraction count[b] = sum_k M[k] * A[k, b] ----
    psum_ca = psum_pool.tile([128, 1], F32)
    psum_cb = psum_pool.tile([NBINS - 128, 1], F32)
    for cquad in range(6):
        nc.tensor.matmul(
            out=psum_ca[:], lhsT=a_sb[:, cquad, 0:128], rhs=m_rhs[:, cquad : cquad + 1],
            start=(cquad == 0), stop=(cquad == 5),
        )
    for cquad in range(6):
        nc.tensor.matmul(
            out=psum_cb[:], lhsT=a_sb[:, cquad, 128:NBINS], rhs=m_rhs[:, cquad : cquad + 1],
            start=(cquad == 0), stop=(cquad == 5),
        )
    cnt = const_pool.tile([128, 2], F32)
    nc.vector.memset(cnt[:], 0.0)
    nc.vector.tensor_copy(out=cnt[:, 0:1], in_=psum_ca[:])
    nc.vector.tensor_copy(out=cnt[0 : NBINS - 128, 1:2], in_=psum_cb[:])

    # total + normalize
    psum_t = psum_pool.tile([1, 2], F32)
    nc.tensor.matmul(out=psum_t[:], lhsT=ones_col[:], rhs=cnt[:], start=True, stop=True)
    tt = const_pool.tile([1, 2], F32)
    nc.vector.tensor_copy(out=tt[:], in_=psum_t[:])
    tot = const_pool.tile([1, 1], F32)
    nc.vector.tensor_tensor(out=tot[:], in0=tt[:, 0:1], in1=tt[:, 1:2], op=OP.add)
    rtot = const_pool.tile([1, 1], F32)
    nc.vector.reciprocal(rtot[:], tot[:])
    psum_r = psum_pool.tile([128, 1], F32)
    nc.tensor.matmul(out=psum_r[:], lhsT=ones_row[:], rhs=rtot[:], start=True, stop=True)
    outn = const_pool.tile([128, 2], F32)
    nc.vector.tensor_tensor(
        out=outn[:], in0=cnt[:], in1=psum_r[:].to_broadcast([128, 2]), op=OP.mult
    )
    nc.sync.dma_start(out=out[0:128], in_=outn[:, 0])
    nc.sync.dma_start(out=out[128:NBINS], in_=outn[0 : NBINS - 128, 1])


# ---------------- host side ----------------

def pack_inputs(xyz: np.ndarray, angle_list: np.ndarray, per: int):
    """Pack host-side into ONE per-core blob: packed int32 angle words plus
    this core's xyz frame slice (f32 bits in the last 12 columns).

    Returns blob_global [N_CORES*128, CHUNKS*C+12] int32."""
    a = np.asarray(angle_list)
    if a.dtype == np.int64 and a.flags.c_contiguous:
        a = a.view(np.int32)[:, 0::2]  # little-endian low words, no copy
    elif a.dtype != np.int32:
        a = a.astype(np.int32)
    w = a[:, 3] << 21  # [M] packed word (f | i<<3 | c<<12 | j<<21)
    w |= a[:, 2] << 12
    w |= a[:, 1] << 3
    w |= a[:, 0]
    n_cores = N_CORES
    blob = np.zeros((n_cores, 128, CHUNKS * C + 12), dtype=np.int32)
    wp = np.zeros((n_cores, SLOTS), dtype=np.int32)
    wp[:, :per] = w.reshape(n_cores, per)
    # p-minor slot order: s' = (k*C + j)*128 + p  ->  [128, CHUNKS*C]
    blob[:, :, : CHUNKS * C] = wp.reshape(n_cores, CHUNKS * C, 128).transpose(0, 2, 1)

    # xyz word (c3, a=hi*128+lo) of core r (= frame r) -> blob[r, lo, 512+c3*4+hi]
    xyzs = np.ascontiguousarray(
        np.asarray(xyz, dtype=np.float32).transpose(0, 2, 1)
    ).view(np.int32)  # [8, 3, 512]
    xw = xyzs.reshape(n_cores, 3, 4, 128).transpose(0, 3, 1, 2)  # [r, lo, c3, hi]
    blob[:, :, CHUNKS * C :] = xw.reshape(n_cores, 128, 12)
    return blob.reshape(n_cores * 128, CHUNKS * C + 12)


_PROG_CACHE = {}


def build_program(per=PER_CORE, chunks=CHUNKS, cols=C):
    key = (per, chunks, cols)
    if key in _PROG_CACHE:
        return _PROG_CACHE[key]
    nc = bacc.Bacc("TRN2", target_bir_lowering=False, num_devices=N_CORES)
    ins = {
        "blob": nc.dram_tensor(
            "blob", [128, chunks * cols + 12], I32, kind="ExternalInput"
        ).ap(),
    }
    consts = (
        nc.inline_tensor(build_mask_math(per, chunks, cols), name="maskc"),
        nc.inline_tensor(amat_inline(), name="amatc"),
        nc.inline_tensor(coefs_tile(), name="coefsc"),
    )
    outs = {"out": nc.dram_tensor("out", [NBINS], F32, kind="ExternalOutput").ap()}
    # raw ap_gather buffers: must be allocated BEFORE TileContext so the tile
    # pools (which claim the free SBUF region at entry) don't overlap them.
    xyzt_sb = nc.alloc_sbuf_tensor("xyzt_sb", [128, N_FRAMES * N_ATOMS], F32)
    idxs16_raw = [
        nc.alloc_sbuf_tensor(f"idxs16r{i}", [128, 3 * cols], mybir.dt.int16)
        for i in range(2)
    ]
    gath_raw = [
        nc.alloc_sbuf_tensor(f"gathr{i}", [128, 3 * 16 * cols], F32)
        for i in range(2)
    ]
    raw = (xyzt_sb, idxs16_raw, gath_raw)
    with tile.TileContext(nc) as tc:
        adf_kernel(tc, outs, ins, raw, consts, per=per)
    nc.compile()
    _PROG_CACHE[key] = nc
    return nc


_RUNNER_CACHE = {}


def _get_runner(per=PER_CORE):
    """Build (once) the cached jitted SPMD executor for the bass program."""
    key = per
    if key in _RUNNER_CACHE:
        return _RUNNER_CACHE[key]

    import jax
    from jax.sharding import Mesh, PartitionSpec
    from jax.experimental.shard_map import shard_map
    from concourse import bass2jax

    nc = build_program(per=per)
    bass2jax.install_neuronx_cc_hook()

    partition_name = nc.partition_id_tensor.name if nc.partition_id_tensor else None
    in_names, out_names, out_avals = [], [], []
    for alloc in nc.m.functions[0].allocations:
        if not isinstance(alloc, mybir.MemoryLocationSet):
            continue
        name = alloc.memorylocations[0].name
        if alloc.kind == "ExternalInput":
            if name != partition_name:
                in_names.append(name)
        elif alloc.kind == "ExternalOutput":
            out_names.append(name)
            out_avals.append(
                jax.core.ShapedArray(tuple(alloc.tensor_shape), mybir.dt.np(alloc.dtype))
            )
    assert in_names == ["blob"], in_names
    assert out_names == ["out"], out_names
    all_in = list(in_names) + ([partition_name] if partition_name else [])

    def _body(*args):
        operands = list(args)
        if partition_name:
            operands.append(bass2jax.partition_id_tensor())
        return tuple(
            bass2jax._bass_exec_p.bind(
                *operands,
                out_avals=tuple(out_avals),
                in_names=tuple(all_in),
                out_names=tuple(out_names),
                lowering_input_output_aliases=(),
                sim_require_finite=True,
                sim_require_nnan=True,
                nc=nc,
            )
        )

    devices = jax.devices()[:N_CORES]
    assert len(devices) == N_CORES
    mesh = Mesh(np.asarray(devices), ("core",))
    fn = jax.jit(
        shard_map(
            _body,
            mesh=mesh,
            in_specs=tuple(PartitionSpec("core") for _ in in_names),
            out_specs=tuple(PartitionSpec("core") for _ in out_names),
            check_rep=False,
        )
    )
    from jax.sharding import NamedSharding

    shard = NamedSharding(mesh, PartitionSpec("core"))
    runner = (fn, out_avals, shard)
    _RUNNER_CACHE[key] = runner
    return runner


def kernel(**inputs) -> np.ndarray:
    import time as _time

    xyz = np.asarray(inputs["xyz"], dtype=np.float32)
    angle_list = np.asarray(inputs["angle_list"])
    m = angle_list.shape[0]
    assert m % N_CORES == 0, f"angle count {m} must divide across {N_CORES} cores"
    per = m // N_CORES
    assert per <= SLOTS, f"angle count {m} exceeds kernel capacity"

    fn, out_avals, shard = _get_runner(per=per)
    blob_g = pack_inputs(xyz, angle_list, per)

    import jax

    t0 = _time.time()
    # async put: the transfer starts immediately and overlaps with dispatch
    d_blob = jax.device_put(blob_g, shard)
    outs = fn(d_blob)
    # all cores hold the identical allreduced histogram; fetch core 0's shard
    out0 = np.asarray(outs[0].addressable_shards[0].data)
    kernel._last_run_s = _time.time() - t0
    kernel._last_results = None
    return np.asarray(out0, dtype=np.float32).reshape(NBINS)


if __name__ == "__main__":
    # smoke: build only
    build_program()
    print("program built ok")
